# revision 24
# baseline (speedup 1.0000x reference)
import numpy as np
import ml_dtypes

import concourse.bass as bass
import concourse.bacc as bacc
import concourse.mybir as mybir
import concourse.tile as tile
from concourse import bass_utils

F32 = mybir.dt.float32
F16 = mybir.dt.float16
BF16 = mybir.dt.bfloat16
FP8 = mybir.dt.float8e4
AF = mybir.ActivationFunctionType
ALU = mybir.AluOpType

B, N, HID = 4, 4096, 128
HALF = N // 2          # rows per core
NB = N // 128          # 32 j-blocks (own-half-first order)
NSWEEP = 6             # Jacobi sweeps, all-fp16 matmuls (sweep 1 has none)

# f32 pack layout (columns)
PK_CWT, PK_WIH, PK_EYE, PK_MLO, PK_MHI, PK_XDF, PK_CORR = (
    0, 384, 896, 1024, 1025, 1026, 1156)
PKF32_W = 1668
# f16 pack layout
PK_WHH, PK_EYE16 = 0, 512
PKF16_W = 640

_CACHED = {}


def build_nc(dbg=False):
    nc = bacc.Bacc("TRN2", target_bir_lowering=False, debug=False, num_devices=8)

    atf8 = nc.dram_tensor("atf8", [128, NB * 2048], FP8, kind="ExternalInput")
    x16r = nc.dram_tensor("x16r", [128, N], F16, kind="ExternalInput")
    packf32 = nc.dram_tensor("packf32", [128, PKF32_W], F32, kind="ExternalInput")
    packf16 = nc.dram_tensor("packf16", [128, PKF16_W], F16, kind="ExternalInput")
    out = nc.dram_tensor("out", [HALF, HID], F32, kind="ExternalOutput")
    if dbg:
        lw_dbg = nc.dram_tensor("lw_dbg", [128, 128], F16, kind="ExternalOutput")
        deg_dbg = nc.dram_tensor("deg_dbg", [128, 32], F32, kind="ExternalOutput")
        zx_dbg = nc.dram_tensor("zx_dbg", [128, 512], F32, kind="ExternalOutput")

    with tile.TileContext(nc) as tc:
        with (
            tc.tile_pool(name="const", bufs=1) as cp,
            tc.tile_pool(name="big", bufs=1) as bigp,
            tc.tile_pool(name="sw", bufs=2) as swp,
            tc.tile_pool(name="outs", bufs=3) as osp,
            tc.tile_pool(name="psdeg", bufs=1, space="PSUM") as psb,
            tc.tile_pool(name="psz", bufs=2, space="PSUM") as psz,
            tc.tile_pool(name="pso", bufs=2, space="PSUM") as pso,
            tc.tile_pool(name="dram", bufs=1, space="DRAM") as dram,
        ):
            # ---------- warm up the collective stream with a dummy AllGather ----------
            warm_sb = cp.tile([1, 16], F32, tag="warmsb")
            nc.vector.memset(warm_sb[:], 1.0)
            cc_warm_in = dram.tile([1, 16], F32)
            cc_warm_out = dram.tile([2, 1, 16], F32)
            nc.gpsimd.dma_start(cc_warm_in[:], warm_sb[:])
            nc.gpsimd.collective_compute(
                "AllGather", ALU.bypass,
                replica_groups=[[0, 1], [2, 3], [4, 5], [6, 7]],
                ins=[cc_warm_in.opt()], outs=[cc_warm_out.opt()],
            )

            # ---------- loads ----------
            pf32 = cp.tile([128, PKF32_W], F32, tag="pf32")
            nc.sync.dma_start(pf32[:], packf32[:])
            pf16 = cp.tile([128, PKF16_W], F16, tag="pf16")
            nc.sync.dma_start(pf16[:], packf16[:])

            at_sb = bigp.tile([128, NB * 2048], FP8, tag="at")

            def at_chunk_dma(c):
                nc.sync.dma_start(
                    at_sb[:, c * 8192:(c + 1) * 8192],
                    atf8[:, c * 8192:(c + 1) * 8192],
                )

            for c in range(8):
                at_chunk_dma(c)
            x16_sb = cp.tile([128, N], F16, tag="x16")
            nc.sync.dma_start(x16_sb[:], x16r[:])

            eyef_sb = pf32[:, PK_EYE:PK_EYE + 128]
            mlo_sb = pf32[:, PK_MLO:PK_MLO + 1]
            mhi_sb = pf32[:, PK_MHI:PK_MHI + 1]
            eye16_sb = pf16[:, PK_EYE16:PK_EYE16 + 128]

            ones8 = cp.tile([128, 32], FP8, tag="ones8")
            nc.vector.memset(ones8[:], 1.0)

            # ---------- conv -> dynT[d, t] (xdf pre-padded in the pack) ----------
            dyn_ps = psz.tile([128, 512], F32, tag="zps")
            for k in range(3):
                nc.tensor.matmul(
                    dyn_ps[:, 0:128], pf32[:, PK_XDF + k:PK_XDF + k + 128],
                    pf32[:, PK_CWT + k * 128:PK_CWT + (k + 1) * 128],
                    start=(k == 0), stop=(k == 2),
                )
            dynT_sb = cp.tile([128, 128], F32, tag="dynT")
            nc.vector.tensor_copy(dynT_sb[:], dyn_ps[:, 0:128])

            # ---------- Zx[u, (g,t)] ----------
            Zx_sb = cp.tile([128, 512], F32, tag="Zx")
            zx_ps = psz.tile([128, 512], F32, tag="zps")
            for g in range(4):
                nc.tensor.matmul(
                    zx_ps[:, g * 128:(g + 1) * 128],
                    pf32[:, PK_WIH + g * 128:PK_WIH + (g + 1) * 128],
                    dynT_sb[:], start=True, stop=True,
                )
            for g in range(4):
                zxg = Zx_sb[:, g * 128:(g + 1) * 128]
                nc.vector.tensor_tensor(
                    zxg, zx_ps[:, g * 128:(g + 1) * 128],
                    pf32[:, PK_CORR + g * 128:PK_CORR + (g + 1) * 128], op=ALU.add,
                )

            # ---------- degree accumulators (DoubleRow fp8 ones) ----------
            deg_ps = [
                psb.tile([128, 512], F32, tag=f"deg{i}", name=f"deg_ps{i}")
                for i in range(4)
            ]
            ones8_ap = ones8[:].rearrange("p (ko m) -> p ko m", ko=2)[:, :, 0:1]
            at3 = at_sb[:].rearrange("p (jb x) -> p jb x", jb=NB)

            def deg_chunk_mms(c):
                for pr in range(c * 2, c * 2 + 2):
                    for sb_i in range(4):
                        nc.tensor.matmul(
                            deg_ps[sb_i][0:1, :], ones8_ap,
                            at3[:, 2 * pr:2 * pr + 2, sb_i * 512:(sb_i + 1) * 512],
                            start=(pr == 0), stop=(pr == 15),
                            perf_mode=mybir.MatmulPerfMode.DoubleRow,
                            skip_group_check=True,
                        )

            # ---------- LSTM Jacobi sweeps, deg chunks interleaved on PE ----------
            H16 = cp.tile([128, 129], F16, tag="H16")
            H32 = cp.tile([128, 129], F32, tag="H32")
            nc.vector.memset(H16[:], 0.0)
            nc.vector.memset(H32[:], 0.0)

            deg_row = cp.tile([1, 2048], F32, tag="degrow")
            deg_dram = dram.tile([1, 2048], F32)
            cc_in = dram.tile([1, 2048], F32)
            cc_out = dram.tile([2, 1, 2048], F32)

            # chunk schedule: s1 {0,1}; s2 {2,3}; s3 {4,5}; s4 {6}; s5 {7}+export
            for s in range(1, NSWEEP + 1):
                if s == 1:
                    deg_chunk_mms(0)
                    deg_chunk_mms(1)
                    zsrc = Zx_sb
                else:
                    zps = psz.tile([128, 512], F32, tag="zps")
                    for g in range(4):
                        nc.tensor.matmul(
                            zps[:, g * 128:(g + 1) * 128],
                            pf16[:, PK_WHH + g * 128:PK_WHH + (g + 1) * 128],
                            H16[:, 0:128], start=True, stop=True,
                        )
                    if s == 2:
                        deg_chunk_mms(2)
                        deg_chunk_mms(3)
                    elif s == 3:
                        deg_chunk_mms(4)
                        deg_chunk_mms(5)
                    elif s == 4:
                        deg_chunk_mms(6)
                    elif s == 5:
                        deg_chunk_mms(7)
                    z = swp.tile([128, 512], F32, tag="z")
                    nc.vector.tensor_tensor(z[:, 0:384], zps[:, 0:384], Zx_sb[:, 0:384], op=ALU.add)
                    nc.vector.tensor_tensor(z[:, 384:512], zps[:, 384:512], Zx_sb[:, 384:512], op=ALU.add)
                    zsrc = z
                G = swp.tile([128, 512], F32, tag="G")
                nc.scalar.activation(G[:, 0:384], zsrc[:, 0:384], AF.Sigmoid)
                nc.scalar.activation(G[:, 384:512], zsrc[:, 384:512], AF.Tanh)
                u_t = swp.tile([128, 128], F32, tag="u")
                nc.vector.tensor_tensor(u_t[:], G[:, 0:128], G[:, 384:512], op=ALU.mult)
                Ct = swp.tile([128, 128], F32, tag="C")
                nc.vector.tensor_tensor_scan(
                    Ct[:], G[:, 128:256], u_t[:], 0.0, op0=ALU.mult, op1=ALU.add
                )
                Tt = swp.tile([128, 128], F32, tag="T")
                nc.scalar.activation(Tt[:], Ct[:], AF.Tanh)
                hdst = H16 if s < NSWEEP else H32
                nc.vector.tensor_tensor(hdst[:, 1:129], G[:, 256:384], Tt[:], op=ALU.mult)

                if s == 5:
                    # degrees complete: psum -> sbuf row, local DRAM roundtrip for
                    # own dinv, pair AllGather for the peer half.
                    nc.scalar.copy(deg_row[0:1, 0:512], deg_ps[0][0:1, :])
                    nc.scalar.copy(deg_row[0:1, 512:1024], deg_ps[1][0:1, :])
                    nc.vector.tensor_copy(deg_row[0:1, 1024:1536], deg_ps[2][0:1, :])
                    nc.vector.tensor_copy(deg_row[0:1, 1536:2048], deg_ps[3][0:1, :])
                    nc.sync.dma_start(deg_dram[:], deg_row[:])
                    nc.gpsimd.dma_start(cc_in[:], deg_row[:])
                    nc.gpsimd.collective_compute(
                        "AllGather", ALU.bypass,
                        replica_groups=[[0, 1], [2, 3], [4, 5], [6, 7]],
                        ins=[cc_in.opt()], outs=[cc_out.opt()],
                    )

            # PE warmers: keep the array busy until agg-lo's inputs are ready
            warm_ps = psz.tile([128, 512], F32, tag="zps")
            for w in range(16):
                nc.tensor.matmul(
                    warm_ps[0:1, :], ones8_ap,
                    at3[:, 2 * (w % 8):2 * (w % 8) + 2, 0:512],
                    start=True, stop=True,
                    perf_mode=mybir.MatmulPerfMode.DoubleRow,
                    skip_group_check=True,
                )

            # ---------- own-half dinv (local roundtrip; own j-blocks are 0..15) ----------
            deg_own = cp.tile([128, 16], F32, tag="degown")
            nc.sync.dma_start(
                deg_own[:].rearrange("p (o rb) -> p o rb", o=1),
                deg_dram[:].rearrange("o (rb p) -> p o rb", p=128),
            )
            dinv_all = cp.tile([128, 32], F32, tag="dinva")
            sq_own = cp.tile([128, 16], F32, tag="sqown")
            nc.scalar.activation(sq_own[:], deg_own[:], AF.Sqrt)
            nc.vector.reciprocal(dinv_all[:, 0:16], sq_own[:])

            # Xs (own half) = dinv_j * X
            xs = cp.tile([128, N], F16, tag="xs")
            for jb in range(16):
                nc.vector.tensor_scalar_mul(
                    xs[:, jb * 128:(jb + 1) * 128],
                    x16_sb[:, jb * 128:(jb + 1) * 128],
                    dinv_all[:, jb:jb + 1],
                )

            # ---------- agg lo half (own j-blocks) ----------
            agg_ps = [
                psb.tile([128, 512], F32, tag=f"deg{i}", name=f"agg_ps{i}")
                for i in range(4)
            ]
            for jb in range(16):
                for sb_i in range(4):
                    nc.tensor.matmul(
                        agg_ps[sb_i][:], xs[:, jb * 128:(jb + 1) * 128],
                        at_sb[:, jb * 2048 + sb_i * 512:jb * 2048 + sb_i * 512 + 512],
                        start=(jb == 0), stop=False,
                        skip_group_check=True,
                    )

            # lw[t, u] = H32[:, 1:129]^T
            lw_ps = pso.tile([128, 128], F32, tag="outps")
            nc.tensor.transpose(lw_ps[:], H32[:, 1:129], eyef_sb)
            lw16 = cp.tile([128, 128], F16, tag="lw16")
            nc.vector.tensor_copy(lw16[:], lw_ps[:])

            # ---------- peer-half dinv from the AllGather ----------
            peer_raw = cp.tile([128, 32], F32, tag="peerraw")
            nc.sync.dma_start(
                peer_raw[:, 0:16].rearrange("p (o rb) -> p o rb", o=1),
                cc_out[0].rearrange("o (rb p) -> p o rb", p=128),
            )
            nc.sync.dma_start(
                peer_raw[:, 16:32].rearrange("p (o rb) -> p o rb", o=1),
                cc_out[1].rearrange("o (rb p) -> p o rb", p=128),
            )
            p1 = cp.tile([128, 16], F32, tag="p1")
            p2 = cp.tile([128, 16], F32, tag="p2")
            nc.vector.tensor_scalar_mul(p1[:], peer_raw[:, 0:16], mhi_sb)
            nc.vector.tensor_scalar_mul(p2[:], peer_raw[:, 16:32], mlo_sb)
            peer_deg = cp.tile([128, 16], F32, tag="peerdeg")
            nc.vector.tensor_tensor(peer_deg[:], p1[:], p2[:], op=ALU.add)
            sq_peer = cp.tile([128, 16], F32, tag="sqpeer")
            nc.scalar.activation(sq_peer[:], peer_deg[:], AF.Sqrt)
            nc.vector.reciprocal(dinv_all[:, 16:32], sq_peer[:])
            for jb in range(16, NB):
                nc.vector.tensor_scalar_mul(
                    xs[:, jb * 128:(jb + 1) * 128],
                    x16_sb[:, jb * 128:(jb + 1) * 128],
                    dinv_all[:, jb:jb + 1],
                )

            if dbg:
                nc.sync.dma_start(lw_dbg[:], lw16[:])
                nc.sync.dma_start(deg_dbg[:], dinv_all[:])
                nc.sync.dma_start(zx_dbg[:], Zx_sb[:])

            # ---------- agg hi half + out (per-superblock for tail overlap) ----------
            o_big = cp.tile([128, 2048], F32, tag="obig")
            for sb_i in range(4):
                for jb in range(16, NB):
                    nc.tensor.matmul(
                        agg_ps[sb_i][:], xs[:, jb * 128:(jb + 1) * 128],
                        at_sb[:, jb * 2048 + sb_i * 512:jb * 2048 + sb_i * 512 + 512],
                        start=False, stop=(jb == NB - 1),
                        skip_group_check=True,
                    )
                aggT = osp.tile([128, 512], F16, tag="aggT")
                nc.vector.tensor_copy(aggT[:], agg_ps[sb_i][:])
                for q in range(4):
                    ib = sb_i * 4 + q
                    out_ps = pso.tile([128, 128], F32, tag="outps")
                    nc.tensor.matmul(
                        out_ps[:], aggT[:, q * 128:(q + 1) * 128], lw16[:],
                        start=True, stop=True,
                    )
                    nc.scalar.activation(
                        o_big[:, ib * 128:(ib + 1) * 128], out_ps[:], AF.Sigmoid,
                        scale=dinv_all[:, ib:ib + 1],
                    )
                if sb_i == 1 or sb_i == 3:
                    half = sb_i // 2
                    nc.sync.dma_start(
                        out[half * 1024:(half + 1) * 1024, :].rearrange(
                            "(rb p) f -> p rb f", p=128
                        ),
                        o_big[:, half * 1024:(half + 1) * 1024].rearrange(
                            "p (rb f) -> p rb f", rb=8
                        ),
                    )
    nc.compile()
    return nc


PERM = np.concatenate([np.arange(0, 128), np.arange(128, 256),
                       np.arange(384, 512), np.arange(256, 384)])
# fp8 e4m3 encodings of 0.0, 1.0, 2.0
FP8_LUT = np.array([0x00, 0x38, 0x40], np.uint8)


def kernel(node_embedding, adjacency_matrix, conv_w, conv_b, w_ih, w_hh, b_ih, b_hh):
    if "nc" not in _CACHED:
        _CACHED["nc"] = build_nc()
    nc = _CACHED["nc"]

    X = np.asarray(node_embedding, dtype=np.float32)
    A = np.asarray(adjacency_matrix, dtype=np.float32)
    wih_p = np.asarray(w_ih, dtype=np.float32)[PERM]
    whh_p = np.asarray(w_hh, dtype=np.float32)[PERM]
    bias_p = (np.asarray(b_ih, dtype=np.float32) + np.asarray(b_hh, dtype=np.float32))[PERM]
    conv_b32 = np.asarray(conv_b, dtype=np.float32)

    pf16 = np.zeros((128, PKF16_W), np.float16)
    pf16[:, PK_WHH:PK_WHH + 512] = whh_p.T.astype(np.float16)
    pf16[:, PK_EYE16:PK_EYE16 + 128] = np.eye(128, dtype=np.float16)

    base_pf32 = np.zeros((128, PKF32_W), np.float32)
    base_pf32[:, PK_CWT:PK_CWT + 384] = (
        np.asarray(conv_w, np.float32).transpose(2, 1, 0).transpose(1, 0, 2).reshape(128, 384)
    )
    base_pf32[:, PK_WIH:PK_WIH + 512] = wih_p.T
    base_pf32[:, PK_EYE:PK_EYE + 128] = np.eye(128, dtype=np.float32)
    S = wih_p.reshape(4, 128, 128).sum(axis=2)           # [g, u]
    b4 = bias_p.reshape(4, 128)                          # [g, u]
    corr4 = (S[:, :, None] * conv_b32[None, None, :] + b4[:, :, None])  # [g, u, t]
    base_pf32[:, PK_CORR:PK_CORR + 512] = corr4.transpose(1, 0, 2).reshape(128, 512)

    in_maps = []
    idx = np.arange(HALF)
    for b in range(B):
        Au8 = A[b].astype(np.uint8)
        for h in range(2):
            own = slice(h * HALF, (h + 1) * HALF)
            peer = slice((1 - h) * HALF, (2 - h) * HALF)
            rows_order = np.r_[own, peer]
            ATu8 = Au8[own, :].T[rows_order]
            ATu8 = np.ascontiguousarray(
                ATu8.reshape(NB, 128, HALF).transpose(1, 0, 2).reshape(128, NB * HALF)
            )
            pp = idx % 128
            cols = (idx // 128) * HALF + idx
            ATu8[pp, cols] += 1
            Xp = X[b][rows_order]
            x16r = np.ascontiguousarray(
                Xp.reshape(NB, 128, 128).transpose(1, 0, 2).reshape(128, N)
            ).astype(np.float16)
            pf32 = base_pf32.copy()
            pf32[:, PK_XDF + 1:PK_XDF + 129] = X[b, N - HID:, :]
            pf32[:, PK_MLO] = 1.0 if h == 0 else 0.0
            pf32[:, PK_MHI] = 0.0 if h == 0 else 1.0
            m = {
                "atf8": FP8_LUT[ATu8].view(ml_dtypes.float8_e4m3),
                "x16r": x16r,
                "packf32": pf32,
                "packf16": pf16,
            }
            in_maps.append(m)

    _CACHED["in_maps"] = in_maps
    res = bass_utils.run_bass_kernel_spmd(nc, in_maps, core_ids=list(range(8)))

    out = np.empty((B, N, HID), np.float32)
    for c in range(8):
        b, h = c // 2, c % 2
        out[b, h * HALF:(h + 1) * HALF, :] = res.results[c]["out"]
    return out
